# revision 27
# baseline (speedup 1.0000x reference)
"""Causal single-head attention on 8 Trainium2 NeuronCores.

Problem: x [4, 2048, 1024], w_q/w_k/w_v [1024, 1024] (nn.Linear convention,
y = x @ W.T). Computes q,k,v projections, causal softmax(q k^T / sqrt(D)) @ v.

Weight-fusion restructuring: scores = (x Wq^T)(x Wk^T)^T = x (Wq^T Wk) x^T,
so with M := Wq^T Wk folded on the host (weight-only preprocessing), the
device computes Q' = x_q M and scores = Q' x^T against the RAW x^T — the K
projection disappears. Likewise out = P V = (P x) Wv^T, so the device
computes U = P x and one small projection O = U Wv^T — the V projection
disappears. Per-core tensor work drops ~40% vs the direct formulation.

Sharding: 2 cores per batch element. The 16 query tiles (128 queries each)
have causal kv-prefix lengths; kv work in 256-token supertiles is
ceil((g+1)/2) for tile g. Core parity p takes tiles g = 2k-2+p for k=1..8:
every core has one query tile per kv-length class k (window 256*k tokens) —
one static SPMD program, perfectly balanced. Padding + causal diagonal are
handled by a host-supplied additive mask [128, 256] on the last supertile.

All matmul operands are float16 (11-bit mantissa, full PE rate at any moving
width); softmax statistics and PSUM accumulation stay fp32. Slots are
processed in pairs (8,4),(7,3),(6,2),(5,1); the first pair's score matmuls
are j-interleaved with the second half of the Q' projection so the PE
consumes x^T tiles as the DMA delivers them, and each slot's softmax chain
runs on ACT/DVE underneath the next slot's PE work (2-stage pipeline).
"""
import numpy as np
from contextlib import ExitStack

import concourse.bass as bass
import concourse.tile as tile
import concourse.mybir as mybir
from concourse.masks import make_identity
from concourse.bass_utils import run_bass_kernel_spmd


F32 = mybir.dt.float32
F16 = mybir.dt.float16
AF = mybir.ActivationFunctionType
AX = mybir.AxisListType

B, S, E, D = 4, 2048, 1024, 1024
NCORES = 8
NSLOT = 8              # kv-length classes k=1..8, window = 256*k tokens
NQ = NSLOT * 128       # queries per core
EC = E // 128          # 128-chunks of the embedding dim
TC = S // 128          # 128-chunks of the token dim
SCALE = 1.0 / 32.0     # 1/sqrt(D)
MASKVAL = -30000.0

# processing order: pairs (big, small) with ngr sums <= 6 so the big slot's
# 4 psum banks + small slot's 2 coexist; ends on slot 1 (shortest tail)
SLOT_ORDER = [8, 4, 7, 3, 6, 2, 5, 1]

_prog = None


def _split_multi_waits(nc, max_waits=1):
    """The walrus build in this container has one sync-wait slot per
    instruction; hoist extra waits onto preceding same-engine NoOps."""
    n = 0
    for f in nc.m.functions:
        for b in f.blocks:
            insts = b.instructions
            out = []
            changed = False
            for ins in insts:
                si = ins.sync_info
                if si is not None and len(si.on_wait) > max_waits:
                    waits = list(si.on_wait)
                    for w in waits[:-max_waits]:
                        nop = mybir.InstNoOp(name=f"I-waitsplit-{n}")
                        n += 1
                        nop.engine = ins.engine
                        nop.sync_info = mybir.SyncInfo(on_wait=[w], on_update=[])
                        out.append(nop)
                    ins.sync_info = mybir.SyncInfo(
                        on_wait=waits[-max_waits:], on_update=list(si.on_update))
                    changed = True
                out.append(ins)
            if changed:
                b.instructions = out
    return nc


def _build(split=True):
    nc = bass.Bass(trn_type="TRN2", target_bir_lowering=False, debug=False)
    kTd = nc.dram_tensor("kT", [E, S], F16, kind="ExternalInput").ap()
    xfd = nc.dram_tensor("xf", [S, E], F16, kind="ExternalInput").ap()
    xqd = nc.dram_tensor("xqT", [E, NQ], F16, kind="ExternalInput").ap()
    mTd = nc.dram_tensor("mT", [E, E], F16, kind="ExternalInput").ap()
    wvd = nc.dram_tensor("wvT", [E, D], F16, kind="ExternalInput").ap()
    maskin = nc.dram_tensor("mask", [128, 256], F32, kind="ExternalInput").ap()
    out = nc.dram_tensor("out", [NQ, D], F32, kind="ExternalOutput").ap()

    with tile.TileContext(nc) as tc, ExitStack() as ctx:
        const = ctx.enter_context(tc.tile_pool(name="const", bufs=1))
        ident16 = const.tile([128, 128], F16)
        make_identity(nc, ident16[:])
        mask_sb = const.tile([128, 256], F32)
        # PE warmup: dummy matmuls during the DMA head so the tensor engine
        # is past its DVFS ramp when the first real operand tile lands
        warm = const.tile([128, 512], F16)
        nc.gpsimd.memset(warm[:], 0.0)
        with tc.tile_pool(name="wps", bufs=1, space="PSUM") as wpp:
            wp_ps = wpp.tile([128, 512], F32, name="warmps")
            for r in range(16):
                nc.tensor.matmul(wp_ps[:], warm[:, :128], warm[:],
                                 start=(r == 0), stop=(r == 15))

        ktp = ctx.enter_context(tc.tile_pool(name="ktp", bufs=1))
        kts = [ktp.tile([128, S], F16, name=f"kt{j}") for j in range(EC)]
        xwp = ctx.enter_context(tc.tile_pool(name="xwp", bufs=1))
        xts = [xwp.tile([128, E], F16, name=f"xt{c}") for c in range(TC)]
        wv = [xwp.tile([128, D], F16, name=f"wv{c}") for c in range(EC)]
        qtp = ctx.enter_context(tc.tile_pool(name="qtp", bufs=1))
        qt = [qtp.tile([128, NQ], F16, name=f"qt{j}") for j in range(EC)]
        mp = ctx.enter_context(tc.tile_pool(name="mp", bufs=1))
        mt = [mp.tile([128, E], F16, name=f"mt{i}") for i in range(EC)]
        xqs = [mp.tile([128, NQ], F16, name=f"xq{i}") for i in range(EC)]

        # DMA priority: (M, xq) pairs feed Q' immediately; x^T tiles next
        # feed the j-interleaved first score pair; then x for U, wv for O.
        nc.sync.dma_start(mt[0][:], mTd[0:128, :])
        nc.sync.dma_start(xqs[0][:], xqd[0:128, :])
        for i in range(1, EC):
            nc.sync.dma_start(mt[i][:], mTd[i * 128:(i + 1) * 128, :])
            nc.sync.dma_start(xqs[i][:], xqd[i * 128:(i + 1) * 128, :])
        for j in range(EC):
            nc.sync.dma_start(kts[j][:], kTd[j * 128:(j + 1) * 128, :])
        nc.sync.dma_start(mask_sb[:], maskin[:])
        for c in range(TC):
            nc.sync.dma_start(xts[c][:], xfd[c * 128:(c + 1) * 128, :])
        for c in range(EC):
            nc.sync.dma_start(wv[c][:], wvd[c * 128:(c + 1) * 128, :])

        # slot geometry (query column s = position in SLOT_ORDER)
        kvlen = {k: 256 * k for k in SLOT_ORDER}
        ngrs = {k: (256 * k + 511) // 512 for k in SLOT_ORDER}
        qcol = {k: s for s, k in enumerate(SLOT_ORDER)}

        att = ctx.enter_context(tc.tile_pool(name="att", bufs=1))

        state = {}
        drained = {}

        def drain(k, s_ps, g0, g1):
            """psum score groups [g0, g1) -> s_sb (+mask on the last one),
            with a per-group running max so the row max is ready with the
            last copy."""
            kv = kvlen[k]
            ngr = ngrs[k]
            if k not in drained:
                s_sb = att.tile([128, S], F32, name=f"s{k}", tag="s", bufs=2)
                mparts = att.tile([128, 4], F32, name=f"mp{k}", tag="mparts",
                                  bufs=2)
                drained[k] = (s_sb, mparts)
            s_sb, mparts = drained[k]
            for g in range(g0, g1):
                w = min(512, kv - g * 512)
                if g == ngr - 1:
                    if w == 512:
                        nc.scalar.copy(s_sb[:, kv - 512:kv - 256],
                                       s_ps[g][:, :256])
                        nc.vector.tensor_add(s_sb[:, kv - 256:kv],
                                             s_ps[g][:, 256:512], mask_sb[:])
                    else:
                        nc.vector.tensor_add(s_sb[:, kv - 256:kv],
                                             s_ps[g][:, :256], mask_sb[:])
                else:
                    nc.scalar.copy(s_sb[:, g * 512:(g + 1) * 512], s_ps[g][:])
                nc.vector.reduce_max(mparts[:, g:g + 1],
                                     s_sb[:, g * 512:g * 512 + w], axis=AX.X)

        def softmax(k):
            """running maxes -> row max, exp, 1/l (reads s_sb only)."""
            kv = kvlen[k]
            ngr = ngrs[k]
            s_sb, mparts = drained.pop(k)
            m = att.tile([128, 1], F32, name=f"m{k}", tag="m", bufs=2)
            nc.vector.reduce_max(m[:], mparts[:, :ngr], axis=AX.X)
            negm = att.tile([128, 1], F32, name=f"negm{k}", tag="negm", bufs=2)
            nc.scalar.mul(negm[:], m[:], -SCALE)
            p_sb = att.tile([128, S], F16, name=f"p{k}", tag="p", bufs=2)
            lparts = att.tile([128, 4], F32, name=f"lp{k}", tag="lp", bufs=2)
            for g in range(ngr):
                w = min(512, kv - g * 512)
                nc.scalar.activation(p_sb[:, g * 512:g * 512 + w],
                                     s_sb[:, g * 512:g * 512 + w], AF.Exp,
                                     bias=negm[:], scale=SCALE,
                                     accum_out=lparts[:, g:g + 1])
            lsum = att.tile([128, 1], F32, name=f"ls{k}", tag="ls", bufs=2)
            nc.vector.reduce_sum(lsum[:], lparts[:, :ngr], axis=AX.X)
            linv = att.tile([128, 1], F32, name=f"li{k}", tag="li", bufs=2)
            nc.vector.reciprocal(linv[:], lsum[:])
            state[k] = (p_sb, linv)

        def back(k, pool):
            """P^T, U = P x, U^T, O = U wv^T / l, store."""
            kv = kvlen[k]
            nch = kv // 128
            p_sb, linv = state.pop(k)
            # P^T chunks [t, q] via PE transpose (f16 in -> f16 psum)
            pt = att.tile([128, S], F16, name=f"pt{k}", tag="pt", bufs=2)
            for c in range(nch):
                tps = pool.tile([128, 128], F16, name=f"tp{k}_{c}", tag="tps",
                                bufs=2)
                nc.tensor.transpose(tps[:], p_sb[:, c * 128:(c + 1) * 128],
                                    ident16[:])
                nc.vector.tensor_copy(pt[:, c * 128:(c + 1) * 128], tps[:])
            # U = P x  [q=128, E], moving free 512; h-outer so the first
            # half's psum drains while the second half accumulates
            u_sb = att.tile([128, E], F16, name=f"u{k}", tag="u", bufs=2)
            for h in range(2):
                u_ps = pool.tile([128, 512], F32, name=f"up{k}_{h}",
                                 tag="vps", bufs=3)
                for c in range(nch):
                    nc.tensor.matmul(u_ps[:], pt[:, c * 128:(c + 1) * 128],
                                     xts[c][:, h * 512:(h + 1) * 512],
                                     start=(c == 0), stop=(c == nch - 1))
                nc.vector.tensor_copy(u_sb[:, h * 512:(h + 1) * 512],
                                      u_ps[:])
            # U^T chunks [e, q] via PE transpose (shares the tps tag FIFO)
            ut = att.tile([128, E], F16, name=f"ut{k}", tag="ut", bufs=2)
            for c in range(EC):
                tps = pool.tile([128, 128], F16, name=f"tu{k}_{c}", tag="tps",
                                bufs=2)
                nc.tensor.transpose(tps[:], u_sb[:, c * 128:(c + 1) * 128],
                                    ident16[:])
                nc.scalar.copy(ut[:, c * 128:(c + 1) * 128], tps[:])
            # O = U wv^T, scaled by 1/l on the psum->sbuf copy; h-outer so
            # the first output half scales + stores under the second half
            o_sb = att.tile([128, D], F32, name=f"o{k}", tag="o", bufs=2)
            s = qcol[k]
            for h in range(2):
                o_ps = pool.tile([128, 512], F32, name=f"op{k}_{h}",
                                 tag="vps", bufs=3)
                for c in range(EC):
                    nc.tensor.matmul(o_ps[:], ut[:, c * 128:(c + 1) * 128],
                                     wv[c][:, h * 512:(h + 1) * 512],
                                     start=(c == 0), stop=(c == EC - 1))
                nc.vector.tensor_scalar_mul(o_sb[:, h * 512:(h + 1) * 512],
                                            o_ps[:], linv[:])
                nc.sync.dma_start(out[s * 128:(s + 1) * 128,
                                      h * 512:(h + 1) * 512],
                                  o_sb[:, h * 512:(h + 1) * 512])

        def scores(k, pool, tag):
            """Full score matmul for one slot, j-outer in 2-group halves so
            only 2 psum banks per tag are live; each half drains to s_sb
            (ACT/DVE) while the next half's matmuls run."""
            kv = kvlen[k]
            ngr = ngrs[k]
            sc = qcol[k] * 128
            for g0 in range(0, ngr, 2):
                g1 = min(g0 + 2, ngr)
                s_ps = {g: pool.tile([128, 512], F32, name=f"sp{k}_{g}",
                                     tag=tag, bufs=3) for g in range(g0, g1)}
                for j in range(EC):
                    lhs = qt[j][:, sc:sc + 128]
                    for g in range(g0, g1):
                        w = min(512, kv - g * 512)
                        nc.tensor.matmul(s_ps[g][:, :w], lhs,
                                         kts[j][:, g * 512:g * 512 + w],
                                         start=(j == 0), stop=(j == EC - 1))
                drain(k, s_ps, g0, g1)

        # ---- Phase 1: Q'A (cols 0:512 = slots 8,4,7,3), then j-interleaved
        # [scores(8) | scores(4) | Q'B] so PE consumes kts as DMA lands it.
        k0, k1 = SLOT_ORDER[0], SLOT_ORDER[1]
        with tc.tile_pool(name="ps1", bufs=1, space="PSUM") as pp1:
            for j in range(EC):
                qps = pp1.tile([128, 512], F32, name=f"qa{j}", tag="qps",
                               bufs=2)
                wj = pp1.tile([128, 512], F32, name=f"wa{j}", tag="qps",
                              bufs=2) if j < 6 else None
                for i in range(EC):
                    nc.tensor.matmul(qps[:], mt[i][:, j * 128:(j + 1) * 128],
                                     xqs[i][:, :512], start=(i == 0),
                                     stop=(i == EC - 1))
                    # keep the PE hot while the (mt, xq) stream is the pacer
                    if wj is not None and i in (2, 5):
                        nc.tensor.matmul(wj[:], warm[:, :128], warm[:],
                                         start=(i == 2), stop=(i == 5))
                nc.scalar.copy(qt[j][:, :512], qps[:])
            s_ps0 = [pp1.tile([128, 512], F32, name=f"sp{k0}_{g}", tag="sA",
                              bufs=4) for g in range(ngrs[k0])]
            s_ps1 = [pp1.tile([128, 512], F32, name=f"sp{k1}_{g}", tag="sB",
                              bufs=2) for g in range(ngrs[k1])]
            for j in range(EC):
                lhs = qt[j][:, qcol[k0] * 128:qcol[k0] * 128 + 128]
                for g in range(ngrs[k0]):
                    w = min(512, kvlen[k0] - g * 512)
                    nc.tensor.matmul(s_ps0[g][:, :w], lhs,
                                     kts[j][:, g * 512:g * 512 + w],
                                     start=(j == 0), stop=(j == EC - 1))
                lhs = qt[j][:, qcol[k1] * 128:qcol[k1] * 128 + 128]
                for g in range(ngrs[k1]):
                    w = min(512, kvlen[k1] - g * 512)
                    nc.tensor.matmul(s_ps1[g][:, :w], lhs,
                                     kts[j][:, g * 512:g * 512 + w],
                                     start=(j == 0), stop=(j == EC - 1))
                qps = pp1.tile([128, 512], F32, name=f"qb{j}", tag="qps",
                               bufs=2)
                for i in range(EC):
                    nc.tensor.matmul(qps[:], mt[i][:, j * 128:(j + 1) * 128],
                                     xqs[i][:, 512:], start=(i == 0),
                                     stop=(i == EC - 1))
                nc.vector.tensor_copy(qt[j][:, 512:], qps[:])
            drain(k0, s_ps0, 0, ngrs[k0])
            drain(k1, s_ps1, 0, ngrs[k1])
            softmax(k0)
            softmax(k1)

        # ---- Phase 2: remaining slots, 2-stage pipeline:
        # scores(i) runs on PE while softmax(i-1) runs on ACT/DVE, and
        # back(i-2) fills PE underneath softmax(i).
        with tc.tile_pool(name="ps2", bufs=1, space="PSUM") as pp2:
            for i in range(2, NSLOT):
                k = SLOT_ORDER[i]
                scores(k, pp2, "sps")
                back(SLOT_ORDER[i - 2], pp2)
                softmax(k)
            back(SLOT_ORDER[NSLOT - 2], pp2)
            back(SLOT_ORDER[NSLOT - 1], pp2)
    if split:
        _split_multi_waits(nc)
    return nc


def _masks():
    j = np.arange(256)[None, :]
    i = np.arange(128)[:, None]
    mask0 = np.where(j <= i, 0.0, MASKVAL).astype(np.float32)
    mask1 = np.where(j <= 128 + i, 0.0, MASKVAL).astype(np.float32)
    return mask0, mask1


def _in_maps(x, w_q, w_k, w_v):
    x = np.asarray(x, dtype=np.float32)
    wq = np.asarray(w_q, np.float32)
    wk = np.asarray(w_k, np.float32)
    wv = np.asarray(w_v, np.float32)
    mT = np.ascontiguousarray(wq.T @ wk).astype(np.float16)      # [E, E]
    wvT = np.ascontiguousarray(wv.T).astype(np.float16)          # [E, D]
    mask0, mask1 = _masks()

    in_maps = []
    for c in range(NCORES):
        b, p = divmod(c, 2)
        xb = x[b]                                                # [S, E]
        kT = np.ascontiguousarray(xb.T).astype(np.float16)       # [E, S]
        xf = xb.astype(np.float16)                               # [S, E]
        qrows = np.concatenate(
            [xb[128 * (2 * (k - 1) + p):128 * (2 * (k - 1) + p) + 128, :]
             for k in SLOT_ORDER], axis=0)                       # [NQ, E]
        xqT = np.ascontiguousarray(qrows.T).astype(np.float16)   # [E, NQ]
        in_maps.append({
            "kT": kT, "xf": xf, "xqT": xqT, "mT": mT, "wvT": wvT,
            "mask": mask0 if p == 0 else mask1,
        })
    return in_maps


def _scatter(per_core_out):
    out = np.empty((B, S, D), dtype=np.float32)
    for c in range(NCORES):
        b, p = divmod(c, 2)
        oc = per_core_out[c]                                     # [NQ, D]
        for s, k in enumerate(SLOT_ORDER):
            g = 2 * (k - 1) + p
            out[b, 128 * g:128 * (g + 1), :] = oc[128 * s:128 * (s + 1), :]
    return out


def kernel(x, w_q, w_k, w_v):
    global _prog
    if _prog is None:
        _prog = _build()
    in_maps = _in_maps(x, w_q, w_k, w_v)
    res = run_bass_kernel_spmd(_prog, in_maps, list(range(NCORES)))
    return _scatter([res.results[c]["out"] for c in range(NCORES)])


# revision 33
# speedup vs baseline: 1.0285x; 1.0285x over previous
"""Causal single-head attention on 8 Trainium2 NeuronCores.

Problem: x [4, 2048, 1024], w_q/w_k/w_v [1024, 1024] (nn.Linear convention,
y = x @ W.T). Computes q,k,v projections, causal softmax(q k^T / sqrt(D)) @ v.

Weight-fusion restructuring: scores = (x Wq^T)(x Wk^T)^T = x (Wq^T Wk) x^T,
so with M := Wq^T Wk folded on the host (weight-only preprocessing), the
device computes Q' = x_q M and scores = Q' x^T against the RAW x^T — the K
projection disappears. Likewise out = P V = (P x) Wv^T, so the device
computes U = P x and one small projection O = U Wv^T — the V projection
disappears. Per-core tensor work drops ~40% vs the direct formulation.

Sharding: 2 cores per batch element. The 16 query tiles (128 queries each)
have causal kv-prefix lengths; kv work in 256-token supertiles is
ceil((g+1)/2) for tile g. Core parity p takes tiles g = 2k-2+p for k=1..8:
every core has one query tile per kv-length class k (window 256*k tokens) —
one static SPMD program, perfectly balanced. Padding + causal diagonal are
handled by a host-supplied additive mask [128, 256] on the last supertile.

All matmul operands are float16 (11-bit mantissa, full PE rate at any moving
width); softmax statistics and PSUM accumulation stay fp32. Slots are
processed in pairs (8,4),(7,3),(6,2),(5,1); the first pair's score matmuls
are j-interleaved with the second half of the Q' projection so the PE
consumes x^T tiles as the DMA delivers them, and each slot's softmax chain
runs on ACT/DVE underneath the next slot's PE work (2-stage pipeline).
"""
import numpy as np
from contextlib import ExitStack

import concourse.bass as bass
import concourse.tile as tile
import concourse.mybir as mybir
from concourse.masks import make_identity
from concourse.bass_utils import run_bass_kernel_spmd


F32 = mybir.dt.float32
F16 = mybir.dt.float16
AF = mybir.ActivationFunctionType
AX = mybir.AxisListType

B, S, E, D = 4, 2048, 1024, 1024
NCORES = 8
NSLOT = 8              # kv-length classes k=1..8, window = 256*k tokens
NQ = NSLOT * 128       # queries per core
EC = E // 128          # 128-chunks of the embedding dim
TC = S // 128          # 128-chunks of the token dim
SCALE = 1.0 / 32.0     # 1/sqrt(D)
MASKVAL = -30000.0

# processing order: pairs (big, small) with ngr sums <= 6 so the big slot's
# 4 psum banks + small slot's 2 coexist; ends on slot 1 (shortest tail)
SLOT_ORDER = [8, 4, 7, 3, 6, 2, 5, 1]

_prog = None


def _split_multi_waits(nc, max_waits=1):
    """The walrus build in this container has one sync-wait slot per
    instruction; hoist extra waits onto preceding same-engine NoOps."""
    n = 0
    for f in nc.m.functions:
        for b in f.blocks:
            insts = b.instructions
            out = []
            changed = False
            for ins in insts:
                si = ins.sync_info
                if si is not None and len(si.on_wait) > max_waits:
                    waits = list(si.on_wait)
                    for w in waits[:-max_waits]:
                        nop = mybir.InstNoOp(name=f"I-waitsplit-{n}")
                        n += 1
                        nop.engine = ins.engine
                        nop.sync_info = mybir.SyncInfo(on_wait=[w], on_update=[])
                        out.append(nop)
                    ins.sync_info = mybir.SyncInfo(
                        on_wait=waits[-max_waits:], on_update=list(si.on_update))
                    changed = True
                out.append(ins)
            if changed:
                b.instructions = out
    return nc


def _build(split=True):
    nc = bass.Bass(trn_type="TRN2", target_bir_lowering=False, debug=False)
    kTd = nc.dram_tensor("kT", [E, S], F16, kind="ExternalInput").ap()
    xfd = nc.dram_tensor("xf", [S, E], F16, kind="ExternalInput").ap()
    xqd = nc.dram_tensor("xqT", [E, NQ], F16, kind="ExternalInput").ap()
    mTd = nc.dram_tensor("mT", [E, E], F16, kind="ExternalInput").ap()
    wvd = nc.dram_tensor("wvT", [E, D], F16, kind="ExternalInput").ap()
    maskin = nc.dram_tensor("mask", [128, 256], F32, kind="ExternalInput").ap()
    out = nc.dram_tensor("out", [NQ, D], F32, kind="ExternalOutput").ap()

    with tile.TileContext(nc) as tc, ExitStack() as ctx:
        const = ctx.enter_context(tc.tile_pool(name="const", bufs=1))
        ident16 = const.tile([128, 128], F16)
        make_identity(nc, ident16[:])
        mask_sb = const.tile([128, 256], F32)
        # PE warmup: dummy matmuls during the DMA head so the tensor engine
        # is past its DVFS ramp when the first real operand tile lands
        warm = const.tile([128, 512], F16)
        nc.gpsimd.memset(warm[:], 0.0)
        with tc.tile_pool(name="wps", bufs=1, space="PSUM") as wpp:
            wp_ps = wpp.tile([128, 512], F32, name="warmps")
            for r in range(16):
                nc.tensor.matmul(wp_ps[:], warm[:, :128], warm[:],
                                 start=(r == 0), stop=(r == 15))

        ktp = ctx.enter_context(tc.tile_pool(name="ktp", bufs=1))
        kts = [ktp.tile([128, S], F16, name=f"kt{j}") for j in range(EC)]
        xwp = ctx.enter_context(tc.tile_pool(name="xwp", bufs=1))
        xts = [xwp.tile([128, E], F16, name=f"xt{c}") for c in range(TC)]
        wv = [xwp.tile([128, D], F16, name=f"wv{c}") for c in range(EC)]
        qtp = ctx.enter_context(tc.tile_pool(name="qtp", bufs=1))
        qt = [qtp.tile([128, NQ], F16, name=f"qt{j}") for j in range(EC)]
        mp = ctx.enter_context(tc.tile_pool(name="mp", bufs=1))
        mt = [mp.tile([128, E], F16, name=f"mt{i}") for i in range(EC)]
        xqs = [mp.tile([128, NQ], F16, name=f"xq{i}") for i in range(EC)]

        # DMA priority: (M, xq) pairs feed Q' immediately; x^T tiles next
        # feed the j-interleaved first score pair; then x for U, wv for O.
        nc.sync.dma_start(mt[0][:], mTd[0:128, :])
        nc.sync.dma_start(xqs[0][:], xqd[0:128, :])
        for i in range(1, EC):
            nc.sync.dma_start(mt[i][:], mTd[i * 128:(i + 1) * 128, :])
            nc.sync.dma_start(xqs[i][:], xqd[i * 128:(i + 1) * 128, :])
        for j in range(EC):
            nc.sync.dma_start(kts[j][:], kTd[j * 128:(j + 1) * 128, :])
        nc.sync.dma_start(mask_sb[:], maskin[:])
        for c in range(TC):
            nc.sync.dma_start(xts[c][:], xfd[c * 128:(c + 1) * 128, :])
        for c in range(EC):
            nc.sync.dma_start(wv[c][:], wvd[c * 128:(c + 1) * 128, :])

        # slot geometry (query column s = position in SLOT_ORDER)
        kvlen = {k: 256 * k for k in SLOT_ORDER}
        ngrs = {k: (256 * k + 511) // 512 for k in SLOT_ORDER}
        qcol = {k: s for s, k in enumerate(SLOT_ORDER)}

        att = ctx.enter_context(tc.tile_pool(name="att", bufs=1))

        state = {}
        drained = {}

        def drain(k, s_ps, g0, g1):
            """psum score groups [g0, g1) -> s_sb (+mask on the last one),
            with a per-group running max so the row max is ready with the
            last copy."""
            kv = kvlen[k]
            ngr = ngrs[k]
            if k not in drained:
                s_sb = att.tile([128, S], F32, name=f"s{k}", tag="s", bufs=2)
                mparts = att.tile([128, 4], F32, name=f"mp{k}", tag="mparts",
                                  bufs=2)
                drained[k] = (s_sb, mparts)
            s_sb, mparts = drained[k]
            for g in range(g0, g1):
                w = min(512, kv - g * 512)
                if g == ngr - 1:
                    if w == 512:
                        nc.scalar.copy(s_sb[:, kv - 512:kv - 256],
                                       s_ps[g][:, :256])
                        nc.vector.tensor_add(s_sb[:, kv - 256:kv],
                                             s_ps[g][:, 256:512], mask_sb[:])
                    else:
                        nc.vector.tensor_add(s_sb[:, kv - 256:kv],
                                             s_ps[g][:, :256], mask_sb[:])
                elif g % 2 == 0:
                    # alternate engines so consecutive group drains overlap
                    nc.scalar.copy(s_sb[:, g * 512:(g + 1) * 512], s_ps[g][:])
                else:
                    nc.vector.tensor_copy(s_sb[:, g * 512:(g + 1) * 512],
                                          s_ps[g][:])
                nc.vector.reduce_max(mparts[:, g:g + 1],
                                     s_sb[:, g * 512:g * 512 + w], axis=AX.X)

        def softmax(k):
            """running maxes -> row max, exp, 1/l (reads s_sb only)."""
            kv = kvlen[k]
            ngr = ngrs[k]
            s_sb, mparts = drained.pop(k)
            m = att.tile([128, 1], F32, name=f"m{k}", tag="m", bufs=2)
            nc.vector.reduce_max(m[:], mparts[:, :ngr], axis=AX.X)
            negm = att.tile([128, 1], F32, name=f"negm{k}", tag="negm", bufs=2)
            nc.scalar.mul(negm[:], m[:], -SCALE)
            p_sb = att.tile([128, S], F16, name=f"p{k}", tag="p", bufs=2)
            lparts = att.tile([128, 4], F32, name=f"lp{k}", tag="lp", bufs=2)
            for g in range(ngr):
                w = min(512, kv - g * 512)
                nc.scalar.activation(p_sb[:, g * 512:g * 512 + w],
                                     s_sb[:, g * 512:g * 512 + w], AF.Exp,
                                     bias=negm[:], scale=SCALE,
                                     accum_out=lparts[:, g:g + 1])
            lsum = att.tile([128, 1], F32, name=f"ls{k}", tag="ls", bufs=2)
            nc.vector.reduce_sum(lsum[:], lparts[:, :ngr], axis=AX.X)
            linv = att.tile([128, 1], F32, name=f"li{k}", tag="li", bufs=2)
            nc.vector.reciprocal(linv[:], lsum[:])
            state[k] = (p_sb, linv)

        def back(k, pool):
            """P^T, U = P x, U^T, O = U wv^T / l, store."""
            kv = kvlen[k]
            nch = kv // 128
            p_sb, linv = state.pop(k)
            # P^T chunks [t, q] via PE transpose (f16 in -> f16 psum)
            pt = att.tile([128, S], F16, name=f"pt{k}", tag="pt", bufs=2)
            for c in range(nch):
                tps = pool.tile([128, 128], F16, name=f"tp{k}_{c}", tag="tps",
                                bufs=2)
                nc.tensor.transpose(tps[:], p_sb[:, c * 128:(c + 1) * 128],
                                    ident16[:])
                nc.vector.tensor_copy(pt[:, c * 128:(c + 1) * 128], tps[:])
            # U = P x  [q=128, E], moving free 512; h-outer so the first
            # half's psum drains while the second half accumulates
            u_sb = att.tile([128, E], F16, name=f"u{k}", tag="u", bufs=2)
            for h in range(2):
                u_ps = pool.tile([128, 512], F32, name=f"up{k}_{h}",
                                 tag="vps", bufs=2)
                for c in range(nch):
                    nc.tensor.matmul(u_ps[:], pt[:, c * 128:(c + 1) * 128],
                                     xts[c][:, h * 512:(h + 1) * 512],
                                     start=(c == 0), stop=(c == nch - 1))
                nc.vector.tensor_copy(u_sb[:, h * 512:(h + 1) * 512],
                                      u_ps[:])
            # U^T chunks [e, q] via PE transpose (f16 in -> f16 psum)
            ut = att.tile([128, E], F16, name=f"ut{k}", tag="ut", bufs=2)
            for c in range(EC):
                tps = pool.tile([128, 128], F16, name=f"tu{k}_{c}", tag="tus",
                                bufs=2)
                nc.tensor.transpose(tps[:], u_sb[:, c * 128:(c + 1) * 128],
                                    ident16[:])
                nc.scalar.copy(ut[:, c * 128:(c + 1) * 128], tps[:])
            # O = U wv^T, scaled by 1/l on the psum->sbuf copy; h-outer so
            # the first output half scales + stores under the second half
            o_sb = att.tile([128, D], F32, name=f"o{k}", tag="o", bufs=2)
            s = qcol[k]
            for h in range(2):
                o_ps = pool.tile([128, 512], F32, name=f"op{k}_{h}",
                                 tag="vps", bufs=2)
                for c in range(EC):
                    nc.tensor.matmul(o_ps[:], ut[:, c * 128:(c + 1) * 128],
                                     wv[c][:, h * 512:(h + 1) * 512],
                                     start=(c == 0), stop=(c == EC - 1))
                nc.vector.tensor_scalar_mul(o_sb[:, h * 512:(h + 1) * 512],
                                            o_ps[:], linv[:])
                nc.sync.dma_start(out[s * 128:(s + 1) * 128,
                                      h * 512:(h + 1) * 512],
                                  o_sb[:, h * 512:(h + 1) * 512])

        def scores(k, pool, tag):
            """Full score matmul for one slot, j-outer in 2-group halves so
            only 2 psum banks per tag are live; each half drains to s_sb
            (ACT/DVE) while the next half's matmuls run."""
            kv = kvlen[k]
            ngr = ngrs[k]
            sc = qcol[k] * 128
            for g0 in range(0, ngr, 2):
                g1 = min(g0 + 2, ngr)
                s_ps = {g: pool.tile([128, 512], F32, name=f"sp{k}_{g}",
                                     tag=tag, bufs=2) for g in range(g0, g1)}
                for j in range(EC):
                    lhs = qt[j][:, sc:sc + 128]
                    for g in range(g0, g1):
                        w = min(512, kv - g * 512)
                        nc.tensor.matmul(s_ps[g][:, :w], lhs,
                                         kts[j][:, g * 512:g * 512 + w],
                                         start=(j == 0), stop=(j == EC - 1))
                drain(k, s_ps, g0, g1)

        # ---- Phase 1: Q'A (cols 0:512 = slots 8,4,7,3), then j-interleaved
        # [scores(8) | scores(4) | Q'B] so PE consumes kts as DMA lands it.
        k0, k1 = SLOT_ORDER[0], SLOT_ORDER[1]
        with tc.tile_pool(name="ps1", bufs=1, space="PSUM") as pp1:
            for j in range(EC):
                qps = pp1.tile([128, 512], F32, name=f"qa{j}", tag="qps",
                               bufs=2)
                for i in range(EC):
                    nc.tensor.matmul(qps[:], mt[i][:, j * 128:(j + 1) * 128],
                                     xqs[i][:, :512], start=(i == 0),
                                     stop=(i == EC - 1))
                nc.scalar.copy(qt[j][:, :512], qps[:])
            s_ps0 = [pp1.tile([128, 512], F32, name=f"sp{k0}_{g}", tag="sA",
                              bufs=4) for g in range(ngrs[k0])]
            s_ps1 = [pp1.tile([128, 512], F32, name=f"sp{k1}_{g}", tag="sB",
                              bufs=2) for g in range(ngrs[k1])]
            for j in range(EC):
                lhs = qt[j][:, qcol[k0] * 128:qcol[k0] * 128 + 128]
                for g in range(ngrs[k0]):
                    w = min(512, kvlen[k0] - g * 512)
                    nc.tensor.matmul(s_ps0[g][:, :w], lhs,
                                     kts[j][:, g * 512:g * 512 + w],
                                     start=(j == 0), stop=(j == EC - 1))
                lhs = qt[j][:, qcol[k1] * 128:qcol[k1] * 128 + 128]
                for g in range(ngrs[k1]):
                    w = min(512, kvlen[k1] - g * 512)
                    nc.tensor.matmul(s_ps1[g][:, :w], lhs,
                                     kts[j][:, g * 512:g * 512 + w],
                                     start=(j == 0), stop=(j == EC - 1))
                qps = pp1.tile([128, 512], F32, name=f"qb{j}", tag="qps",
                               bufs=2)
                for i in range(EC):
                    nc.tensor.matmul(qps[:], mt[i][:, j * 128:(j + 1) * 128],
                                     xqs[i][:, 512:], start=(i == 0),
                                     stop=(i == EC - 1))
                nc.vector.tensor_copy(qt[j][:, 512:], qps[:])
            drain(k0, s_ps0, 0, ngrs[k0])
            drain(k1, s_ps1, 0, ngrs[k1])
            softmax(k0)
            softmax(k1)

        # ---- Phase 2: remaining slots, 2-stage pipeline:
        # scores(i) runs on PE while softmax(i-1) runs on ACT/DVE, and
        # back(i-2) fills PE underneath softmax(i).
        with tc.tile_pool(name="ps2", bufs=1, space="PSUM") as pp2:
            for i in range(2, NSLOT):
                k = SLOT_ORDER[i]
                scores(k, pp2, "sps")
                back(SLOT_ORDER[i - 2], pp2)
                softmax(k)
            back(SLOT_ORDER[NSLOT - 2], pp2)
            back(SLOT_ORDER[NSLOT - 1], pp2)
    if split:
        _split_multi_waits(nc)
    return nc


def _masks():
    j = np.arange(256)[None, :]
    i = np.arange(128)[:, None]
    mask0 = np.where(j <= i, 0.0, MASKVAL).astype(np.float32)
    mask1 = np.where(j <= 128 + i, 0.0, MASKVAL).astype(np.float32)
    return mask0, mask1


def _in_maps(x, w_q, w_k, w_v):
    x = np.asarray(x, dtype=np.float32)
    wq = np.asarray(w_q, np.float32)
    wk = np.asarray(w_k, np.float32)
    wv = np.asarray(w_v, np.float32)
    mT = np.ascontiguousarray(wq.T @ wk).astype(np.float16)      # [E, E]
    wvT = np.ascontiguousarray(wv.T).astype(np.float16)          # [E, D]
    mask0, mask1 = _masks()

    in_maps = []
    for c in range(NCORES):
        b, p = divmod(c, 2)
        xb = x[b]                                                # [S, E]
        kT = np.ascontiguousarray(xb.T).astype(np.float16)       # [E, S]
        xf = xb.astype(np.float16)                               # [S, E]
        qrows = np.concatenate(
            [xb[128 * (2 * (k - 1) + p):128 * (2 * (k - 1) + p) + 128, :]
             for k in SLOT_ORDER], axis=0)                       # [NQ, E]
        xqT = np.ascontiguousarray(qrows.T).astype(np.float16)   # [E, NQ]
        in_maps.append({
            "kT": kT, "xf": xf, "xqT": xqT, "mT": mT, "wvT": wvT,
            "mask": mask0 if p == 0 else mask1,
        })
    return in_maps


def _scatter(per_core_out):
    out = np.empty((B, S, D), dtype=np.float32)
    for c in range(NCORES):
        b, p = divmod(c, 2)
        oc = per_core_out[c]                                     # [NQ, D]
        for s, k in enumerate(SLOT_ORDER):
            g = 2 * (k - 1) + p
            out[b, 128 * g:128 * (g + 1), :] = oc[128 * s:128 * (s + 1), :]
    return out


def kernel(x, w_q, w_k, w_v):
    global _prog
    if _prog is None:
        _prog = _build()
    in_maps = _in_maps(x, w_q, w_k, w_v)
    res = run_bass_kernel_spmd(_prog, in_maps, list(range(NCORES)))
    return _scatter([res.results[c]["out"] for c in range(NCORES)])
